# revision 13
# baseline (speedup 1.0000x reference)
"""Trainium2 Bass kernel for AudioToTextCrossEntropyLoss.

Math: loss = mean_b [ ln(sum_j exp(x_bj)) - (sum_{j=t_b}^{t_b+p_b} x_bj)/(p_b+1) ]

Sharding: data-parallel over batch — 1024 rows as 128 rows on each of 8
NeuronCores (rows on partitions). The kernel returns the 128 per-sample
losses per core; the host sums 1024 scalars and divides by 1024.

Per-core algorithm, designed around the engine rooflines:
  - x is sent as fp8_e4m3 (host cast): 4 MiB/core, ~13 us of DMA at the
    ~336 GB/s HBM cap instead of 47 us for f32. fp8 quantization of x
    perturbs ln-sum-exp by ~1e-4 rel.
  - The exp+row-sum work (the only O(B*N) compute) is split across THREE
    engines working on disjoint column ranges in parallel:
      * ScalarE (ACT): table exp with per-chunk accumulate (1 elem/cyc/lane)
        on row-major chunks.
      * VectorE (DVE): Schraudolph bit-trick exp — one 2-elem/cyc
        tensor_scalar computes i16(x*128/ln2 + bias), whose bit pattern
        read as bf16 is exp(x)*2^-16.
      * TensorE (PE): reduces the Schraudolph values — these chunks are
        sent TRANSPOSED (columns on partitions, rows on the free axis), so
        a ones-matmul contracts 128 columns at a time; 512-wide moving
        blocks accumulate into one PSUM bank across all chunks. Per-row
        sums land on the PSUM diagonal psum[r, g*128+r], extracted by one
        masked accumulate against an (r - p) iota.
    Chunks of the streams are interleaved in DMA order so all engines
    start ~1 us in and finish together.
  - The ragged window term needs only cols [t_b, t_b+p_b], p<=63: the host
    ships the 80-wide f32 slab at each row's t_b plus a fused f32 mask
    (-1/(p+1) inside the window, 0 outside); one DVE multiply-accumulate
    yields -window_mean. (Replaces 2 full masked passes over 16k cols.)
  - Final: s = sA + K*(sD_act + sD_pe) with K folding the 2^16 scale and
    the Schraudolph calibration; lse = Ln(s) and ps = Identity(lse + wneg)
    on ACT — the act-table pass is overridden so Exp, Ln and Identity all
    come from the natural_log_exp_and_others set: one table load, in the
    prologue. ps is DMA'd out per sample.
"""

import numpy as np
import ml_dtypes

import bass_rust as _bass_rust
import concourse.bacc as bacc
import concourse.mybir as mybir
import concourse.tile as tile
from concourse.bass_utils import run_bass_kernel_spmd
from concourse.hw_specs import get_activation_tables

F32 = mybir.dt.float32
BF16 = mybir.dt.bfloat16
FP8 = mybir.dt.float8e4
I16 = mybir.dt.int16
ALU = mybir.AluOpType
ACTF = mybir.ActivationFunctionType

B, N = 1024, 32768
NCORES = 8
BL = B // NCORES          # 128 rows per core
WIN = 80                  # window slab width (>= max p+1 = 64)
MMN = 512                 # moving free dim per matmul (PE max, 1 PSUM bank)

# Interleaved chunk schedule: (engine, width). "A" = ScalarE table exp on
# row-major chunks, "D" = DVE Schraudolph convert on transposed chunks
# reduced by PE. Small leading chunks start the engines early; widths are
# sized so all engines finish at ~the same time. D widths must be
# multiples of 512.
SCHED = [
    ("A", 1024), ("D", 2048), ("A", 4096), ("D", 4096),
    ("A", 4096), ("D", 4096), ("A", 3072), ("D", 4096),
    ("D", 4096), ("D", 2048),
]
assert sum(w for _, w in SCHED) == N
A_CHUNKS = [(i, w) for i, (e, w) in enumerate(SCHED) if e == "A"]
D_CHUNKS = [(i, w) for i, (e, w) in enumerate(SCHED) if e == "D"]
AMAX = max(w for _, w in A_CHUNKS)
DMAX = max(w for _, w in D_CHUNKS)
assert all(w % MMN == 0 for _, w in D_CHUNKS)

# Schraudolph constants: i16 value e*128+m bitcast as bf16 reads as
# (1+m/128)*2^(e-127) ~= exp(x)*2^-S when t = x*128/ln2 + (127-S)*128 - C.
# C zeroes the mean of the (1+f)*2^-f - 1 mantissa error; +0.5 centers the
# float->int conversion. CALIB (measured against the HW conversion/rounding
# behavior on fp8-quantized N(0,1)) removes the residual bias; K folds it
# with the 2^S scale.
SCALE_S = 16
A_CONST = 128.0 / float(np.log(2.0))
B_CONST = (127 - SCALE_S) * 128.0 - 7.21927 + 0.5
K_COMB = float(2.0 ** SCALE_S) * 0.99601


def _build():
    nc = bacc.Bacc("TRN2", target_bir_lowering=False, debug=False,
                   num_devices=NCORES)
    # x8 is chunk-major in SCHED order: "A" chunks are row-major [128, w]
    # blocks, "D" chunks are transposed [128 cols, w/128 tiles, 128 rows]
    # blocks — either way contiguous, so the shard streams from DRAM in
    # sequential address order.
    x_d = nc.dram_tensor("x8", [BL * N], FP8, kind="ExternalInput").ap()
    # cols 0..WIN-1: window values (f32); cols WIN..2*WIN-1: -1/(p+1) mask
    aux_d = nc.dram_tensor("aux", [BL, 2 * WIN], F32,
                           kind="ExternalInput").ap()
    out_d = nc.dram_tensor("ps", [BL, 1], F32, kind="ExternalOutput").ap()

    a_tot = sum(w for _, w in A_CHUNKS)
    d_tot = sum(w for _, w in D_CHUNKS)
    n_mm = d_tot // MMN

    with tile.TileContext(nc) as tc:
        with (
            tc.tile_pool(name="xp", bufs=1) as xpool,
            tc.tile_pool(name="dumps", bufs=1) as dumps,
            tc.tile_pool(name="small", bufs=1) as small,
            tc.tile_pool(name="ps", bufs=1, space="PSUM") as pspool,
        ):
            xa = xpool.tile([BL, a_tot], FP8, tag="xa")
            xd = xpool.tile([BL, d_tot], FP8, tag="xd")
            aux = small.tile([BL, 2 * WIN], F32, tag="aux")
            ones_t = small.tile([BL, BL], BF16, tag="ones")
            iota_t = small.tile([BL, MMN], F32, tag="iota")
            pA = small.tile([BL, len(A_CHUNKS)], F32, tag="pA")
            fin = small.tile([BL, 8], F32, tag="fin")
            ti0 = dumps.tile([BL, DMAX], I16, tag="ti0")
            ti1 = dumps.tile([BL, DMAX], I16, tag="ti1")
            expd = dumps.tile([BL, AMAX], BF16, tag="expd")
            wdump = dumps.tile([BL, WIN], F32, tag="wdump")
            ddump = dumps.tile([BL, MMN], F32, tag="ddump")
            psum = pspool.tile([BL, MMN], F32, tag="psum")

            wneg = fin[:, 0:1]    # -window_sum/(p+1)
            rA = fin[:, 1:2]      # sum of ACT partials
            rD = fin[:, 2:3]      # sum of PE-reduced Schraudolph values
            s = fin[:, 3:4]       # total sum of exp
            lse = fin[:, 4:5]
            ps = fin[:, 5:6]      # per-sample loss

            # Prologue constants on otherwise-idle engines: the (r - p)
            # iota for the PSUM diagonal extraction, and the all-ones
            # matmul weights.
            nc.gpsimd.iota(iota_t[:], pattern=[[0, MMN // BL], [1, BL]],
                           base=0, channel_multiplier=-1,
                           allow_small_or_imprecise_dtypes=True)
            nc.gpsimd.memset(ones_t[:], 1.0)

            # Per-chunk offsets into the per-engine SBUF tiles
            a_off, d_off, offs = 0, 0, []
            for e, w in SCHED:
                if e == "A":
                    offs.append(a_off)
                    a_off += w
                else:
                    offs.append(d_off)
                    d_off += w

            # Prologue DMAs: first chunk + aux on the scalar ring (issues
            # ~2 us before the sync ring's first issue), rest on sync.
            flat = 0
            for c, (e, w) in enumerate(SCHED):
                src = x_d[flat:flat + w * BL].rearrange("(p w) -> p w", p=BL)
                flat += w * BL
                dst = xa if e == "A" else xd
                eng = nc.scalar if c == 0 else nc.sync
                eng.dma_start(dst[:, offs[c]:offs[c] + w], src)
                if c == 0:
                    nc.scalar.dma_start(aux[:], aux_d[:])

            # Window term (DVE, runs as soon as aux lands):
            # wneg = sum(win * (-1/(p+1) masked))
            nc.vector.scalar_tensor_tensor(
                wdump[:], aux[:, 0:WIN], 1.0, aux[:, WIN:2 * WIN],
                op0=ALU.mult, op1=ALU.mult, accum_out=wneg)

            # ScalarE stream: table exp + accumulate per chunk
            for i, (c, w) in enumerate(A_CHUNKS):
                o = offs[c]
                nc.scalar.activation(expd[:, :w], xa[:, o:o + w], ACTF.Exp,
                                     accum_out=pA[:, i:i + 1])

            # DVE + PE streams: Schraudolph convert (2 elem/cyc), then the
            # PE contracts each 128-column tile with ones weights, 512-wide
            # moving blocks, accumulating every block into the same PSUM
            # bank. ti ping-pongs so chunk c+1's convert overlaps chunk c's
            # matmuls.
            mm = 0
            for j, (c, w) in enumerate(D_CHUNKS):
                o = offs[c]
                ti = (ti0, ti1)[j % 2]
                nc.vector.tensor_scalar(ti[:, :w], xd[:, o:o + w],
                                        A_CONST, B_CONST,
                                        op0=ALU.mult, op1=ALU.add)
                for g in range(w // MMN):
                    nc.tensor.matmul(
                        psum[:], ones_t[:],
                        ti[:, g * MMN:(g + 1) * MMN].bitcast(BF16),
                        start=(mm == 0), stop=(mm == n_mm - 1))
                    mm += 1

            # psum[p, g*128+r] holds sum-of-group-g-tiles for row r; the
            # row-r total needs the diagonal r == p of each group:
            # rD = sum_f psum[p, f] * (iota[p, f] == 0)
            nc.vector.scalar_tensor_tensor(
                ddump[:], iota_t[:], 0.0, psum[:],
                op0=ALU.is_equal, op1=ALU.mult, accum_out=rD)

            # Combine: s = sum(pA) + K * rD; lse = Ln(s) and the final add
            # on ACT (same act table set as Exp — no mid-kernel load)
            nc.vector.tensor_reduce(rA, pA[:], axis=mybir.AxisListType.X,
                                    op=ALU.add)
            nc.vector.scalar_tensor_tensor(s, rD, K_COMB, rA,
                                           op0=ALU.mult, op1=ALU.add)
            nc.scalar.activation(lse, s, ACTF.Ln)
            nc.scalar.activation(ps, lse, ACTF.Identity, bias=wneg)
            nc.scalar.dma_start(out_d[:], ps)

    # Route Exp, Ln (and Identity etc.) to the one table set containing
    # them all (natural_log_exp_and_others): pass the act tables in
    # original order (ids must stay act_info.json indices), empty set 0
    # (a non-empty set 0 attracts a redundant initial load), and drop the
    # combined set's functions from all other sets so the combined set is
    # always the first match.
    def _patched_act_loads():
        tabs = get_activation_tables(nc.m.arch)
        combined = tabs["natural_log_exp_and_others"]
        items = []
        for name, funcs in tabs.items():
            if name == "exp_and_others":
                funcs = set()
            elif name != "natural_log_exp_and_others":
                funcs = funcs - combined
            items.append((name, funcs))
        _bass_rust.insert_act_table_loads(nc, items)

    nc.insert_act_table_loads = _patched_act_loads
    nc.compile()
    return nc


_NC_CACHE = []


def _get_nc():
    if not _NC_CACHE:
        _NC_CACHE.append(_build())
    return _NC_CACHE[0]


def _make_in_maps(inputs, targets, postive_list):
    x = np.ascontiguousarray(np.asarray(inputs, dtype=np.float32))
    t = np.asarray(targets).astype(np.int64)
    p = np.asarray(postive_list).astype(np.int64)

    x8 = x.astype(ml_dtypes.float8_e4m3)

    # window slab + fused -1/(p+1) mask, from the full-precision input
    cols = np.arange(WIN, dtype=np.int64)
    idx = t[:, None] + cols[None, :]                    # [B, WIN]
    win = np.take_along_axis(x, idx, axis=1)            # [B, WIN] f32
    negmask = np.where(cols[None, :] <= p[:, None],
                       -1.0 / (p[:, None] + 1.0), 0.0).astype(np.float32)
    aux = np.concatenate([win, negmask], axis=1)        # [B, 2*WIN]

    in_maps = []
    for i in range(NCORES):
        sl = slice(i * BL, (i + 1) * BL)
        shard = x8[sl]
        parts, off = [], 0
        for e, w in SCHED:
            blk = shard[:, off:off + w]
            if e == "D":
                # [128 rows, w] -> [128 cols, w/128 tiles, 128 rows]
                blk = np.transpose(
                    blk.reshape(BL, w // BL, BL), (2, 1, 0))
            parts.append(np.ascontiguousarray(blk).reshape(-1))
            off += w
        in_maps.append({
            "x8": np.concatenate(parts),
            "aux": np.ascontiguousarray(aux[sl]),
        })
    return in_maps


def _run(inputs, targets, postive_list, trace=False, **kwargs):
    nc = _get_nc()
    in_maps = _make_in_maps(inputs, targets, postive_list)
    res = run_bass_kernel_spmd(nc, in_maps, core_ids=list(range(NCORES)),
                               trace=trace, **kwargs)
    total = np.float64(0.0)
    for i in range(NCORES):
        total += np.asarray(res.results[i]["ps"], dtype=np.float64).sum()
    value = np.float32(total / B)
    return value, res


def kernel(inputs, targets, postive_list):
    value, _ = _run(inputs, targets, postive_list, trace=False)
    return np.array(value, dtype=np.float32)


# revision 15
# speedup vs baseline: 1.0002x; 1.0002x over previous
"""Trainium2 Bass kernel for AudioToTextCrossEntropyLoss.

Math: loss = mean_b [ ln(sum_j exp(x_bj)) - (sum_{j=t_b}^{t_b+p_b} x_bj)/(p_b+1) ]

Sharding: data-parallel over batch — 1024 rows as 128 rows on each of 8
NeuronCores (rows on partitions). The kernel returns the 128 per-sample
losses per core; the host sums 1024 scalars and divides by 1024.

Per-core algorithm, designed around the engine rooflines:
  - x is sent as fp8_e4m3 (host cast): 4 MiB/core, ~13 us of DMA at the
    ~336 GB/s HBM cap instead of 47 us for f32. fp8 quantization of x
    perturbs ln-sum-exp by ~1e-4 rel.
  - The exp+row-sum work (the only O(B*N) compute) is split across THREE
    engines working on disjoint column ranges in parallel:
      * ScalarE (ACT): table exp with per-chunk accumulate (1 elem/cyc/lane)
        on row-major chunks.
      * VectorE (DVE): Schraudolph bit-trick exp — one 2-elem/cyc
        tensor_scalar computes i16(x*128/ln2 + bias), whose bit pattern
        read as bf16 is exp(x)*2^-16.
      * TensorE (PE): reduces the Schraudolph values — these chunks are
        sent TRANSPOSED (columns on partitions, rows on the free axis), so
        a ones-matmul contracts 128 columns at a time; 512-wide moving
        blocks accumulate into one PSUM bank across all chunks. Per-row
        sums land on the PSUM diagonal psum[r, g*128+r], extracted by one
        masked accumulate against an (r - p) iota.
    Chunks of the streams are interleaved in DMA order so all engines
    start ~1 us in and finish together.
  - The ragged window term needs only cols [t_b, t_b+p_b], p<=63: the host
    ships the 80-wide f32 slab at each row's t_b plus a fused f32 mask
    (-1/(p+1) inside the window, 0 outside); one DVE multiply-accumulate
    yields -window_mean. (Replaces 2 full masked passes over 16k cols.)
  - Final: s = sA + K*(sD_act + sD_pe) with K folding the 2^16 scale and
    the Schraudolph calibration; lse = Ln(s) and ps = Identity(lse + wneg)
    on ACT — the act-table pass is overridden so Exp, Ln and Identity all
    come from the natural_log_exp_and_others set: one table load, in the
    prologue. ps is DMA'd out per sample.
"""

import numpy as np
import ml_dtypes

import bass_rust as _bass_rust
import concourse.bacc as bacc
import concourse.mybir as mybir
import concourse.tile as tile
from concourse.bass_utils import run_bass_kernel_spmd
from concourse.hw_specs import get_activation_tables

F32 = mybir.dt.float32
BF16 = mybir.dt.bfloat16
FP8 = mybir.dt.float8e4
I16 = mybir.dt.int16
ALU = mybir.AluOpType
ACTF = mybir.ActivationFunctionType

B, N = 1024, 32768
NCORES = 8
BL = B // NCORES          # 128 rows per core
WIN = 80                  # window slab width (>= max p+1 = 64)
MMN = 512                 # moving free dim per matmul (PE max, 1 PSUM bank)

# Interleaved chunk schedule: (engine, width). "A" = ScalarE table exp on
# row-major chunks, "D" = DVE Schraudolph convert on transposed chunks
# reduced by PE. Small leading chunks start the engines early; widths are
# sized so all engines finish at ~the same time. D widths must be
# multiples of 512.
SCHED = [
    ("A", 1024), ("D", 2048), ("A", 4096), ("D", 4096),
    ("A", 4096), ("D", 4096), ("A", 4096), ("D", 4096),
    ("D", 4096), ("D", 1024),
]
assert sum(w for _, w in SCHED) == N
A_CHUNKS = [(i, w) for i, (e, w) in enumerate(SCHED) if e == "A"]
D_CHUNKS = [(i, w) for i, (e, w) in enumerate(SCHED) if e == "D"]
AMAX = max(w for _, w in A_CHUNKS)
DMAX = max(w for _, w in D_CHUNKS)
assert all(w % MMN == 0 for _, w in D_CHUNKS)

# Schraudolph constants: i16 value e*128+m bitcast as bf16 reads as
# (1+m/128)*2^(e-127) ~= exp(x)*2^-S when t = x*128/ln2 + (127-S)*128 - C.
# C zeroes the mean of the (1+f)*2^-f - 1 mantissa error; +0.5 centers the
# float->int conversion. CALIB (measured against the HW conversion/rounding
# behavior on fp8-quantized N(0,1)) removes the residual bias; K folds it
# with the 2^S scale.
SCALE_S = 16
A_CONST = 128.0 / float(np.log(2.0))
B_CONST = (127 - SCALE_S) * 128.0 - 7.21927 + 0.5
K_COMB = float(2.0 ** SCALE_S) * 0.99601


def _build():
    nc = bacc.Bacc("TRN2", target_bir_lowering=False, debug=False,
                   num_devices=NCORES)
    # x8 is chunk-major in SCHED order: "A" chunks are row-major [128, w]
    # blocks, "D" chunks are transposed [128 cols, w/128 tiles, 128 rows]
    # blocks — either way contiguous, so the shard streams from DRAM in
    # sequential address order.
    x_d = nc.dram_tensor("x8", [BL * N], FP8, kind="ExternalInput").ap()
    # cols 0..WIN-1: window values (f32); cols WIN..2*WIN-1: -1/(p+1) mask
    aux_d = nc.dram_tensor("aux", [BL, 2 * WIN], F32,
                           kind="ExternalInput").ap()
    out_d = nc.dram_tensor("ps", [BL, 1], F32, kind="ExternalOutput").ap()

    a_tot = sum(w for _, w in A_CHUNKS)
    d_tot = sum(w for _, w in D_CHUNKS)
    n_mm = d_tot // MMN

    with tile.TileContext(nc) as tc:
        with (
            tc.tile_pool(name="xp", bufs=1) as xpool,
            tc.tile_pool(name="dumps", bufs=1) as dumps,
            tc.tile_pool(name="small", bufs=1) as small,
            tc.tile_pool(name="ps", bufs=1, space="PSUM") as pspool,
        ):
            xa = xpool.tile([BL, a_tot], FP8, tag="xa")
            xd = xpool.tile([BL, d_tot], FP8, tag="xd")
            aux = small.tile([BL, 2 * WIN], F32, tag="aux")
            ones_t = small.tile([BL, BL], BF16, tag="ones")
            iota_t = small.tile([BL, MMN], F32, tag="iota")
            pA = small.tile([BL, len(A_CHUNKS)], F32, tag="pA")
            fin = small.tile([BL, 8], F32, tag="fin")
            ti0 = dumps.tile([BL, DMAX], I16, tag="ti0")
            ti1 = dumps.tile([BL, DMAX], I16, tag="ti1")
            expd = dumps.tile([BL, AMAX], BF16, tag="expd")
            wdump = dumps.tile([BL, WIN], F32, tag="wdump")
            ddump = dumps.tile([BL, MMN], F32, tag="ddump")
            psum = pspool.tile([BL, MMN], F32, tag="psum")

            wneg = fin[:, 0:1]    # -window_sum/(p+1)
            rA = fin[:, 1:2]      # sum of ACT partials
            rD = fin[:, 2:3]      # sum of PE-reduced Schraudolph values
            s = fin[:, 3:4]       # total sum of exp
            lse = fin[:, 4:5]
            ps = fin[:, 5:6]      # per-sample loss

            # Prologue constants on otherwise-idle engines: the (r - p)
            # iota for the PSUM diagonal extraction, and the all-ones
            # matmul weights.
            nc.gpsimd.iota(iota_t[:], pattern=[[0, MMN // BL], [1, BL]],
                           base=0, channel_multiplier=-1,
                           allow_small_or_imprecise_dtypes=True)
            nc.gpsimd.memset(ones_t[:], 1.0)

            # Per-chunk offsets into the per-engine SBUF tiles
            a_off, d_off, offs = 0, 0, []
            for e, w in SCHED:
                if e == "A":
                    offs.append(a_off)
                    a_off += w
                else:
                    offs.append(d_off)
                    d_off += w

            # Prologue DMAs: first chunk + aux on the scalar ring (issues
            # ~2 us before the sync ring's first issue), rest on sync.
            flat = 0
            for c, (e, w) in enumerate(SCHED):
                src = x_d[flat:flat + w * BL].rearrange("(p w) -> p w", p=BL)
                flat += w * BL
                dst = xa if e == "A" else xd
                eng = nc.scalar if c == 0 else nc.sync
                eng.dma_start(dst[:, offs[c]:offs[c] + w], src)
                if c == 0:
                    nc.sync.dma_start(aux[:], aux_d[:])

            # Window term (DVE, runs as soon as aux lands):
            # wneg = sum(win * (-1/(p+1) masked))
            nc.vector.scalar_tensor_tensor(
                wdump[:], aux[:, 0:WIN], 1.0, aux[:, WIN:2 * WIN],
                op0=ALU.mult, op1=ALU.mult, accum_out=wneg)

            # ScalarE stream: table exp + accumulate per chunk
            for i, (c, w) in enumerate(A_CHUNKS):
                o = offs[c]
                nc.scalar.activation(expd[:, :w], xa[:, o:o + w], ACTF.Exp,
                                     accum_out=pA[:, i:i + 1])

            # DVE + PE streams: Schraudolph convert (2 elem/cyc), then the
            # PE contracts each 128-column tile with ones weights, 512-wide
            # moving blocks, accumulating every block into the same PSUM
            # bank. ti ping-pongs so chunk c+1's convert overlaps chunk c's
            # matmuls.
            mm = 0
            for j, (c, w) in enumerate(D_CHUNKS):
                o = offs[c]
                ti = (ti0, ti1)[j % 2]
                nc.vector.tensor_scalar(ti[:, :w], xd[:, o:o + w],
                                        A_CONST, B_CONST,
                                        op0=ALU.mult, op1=ALU.add)
                for g in range(w // MMN):
                    nc.tensor.matmul(
                        psum[:], ones_t[:],
                        ti[:, g * MMN:(g + 1) * MMN].bitcast(BF16),
                        start=(mm == 0), stop=(mm == n_mm - 1))
                    mm += 1

            # psum[p, g*128+r] holds sum-of-group-g-tiles for row r; the
            # row-r total needs the diagonal r == p of each group:
            # rD = sum_f psum[p, f] * (iota[p, f] == 0)
            nc.vector.scalar_tensor_tensor(
                ddump[:], iota_t[:], 0.0, psum[:],
                op0=ALU.is_equal, op1=ALU.mult, accum_out=rD)

            # Combine: s = sum(pA) + K * rD; lse = Ln(s) and the final add
            # on ACT (same act table set as Exp — no mid-kernel load)
            nc.vector.tensor_reduce(rA, pA[:], axis=mybir.AxisListType.X,
                                    op=ALU.add)
            nc.vector.scalar_tensor_tensor(s, rD, K_COMB, rA,
                                           op0=ALU.mult, op1=ALU.add)
            nc.scalar.activation(lse, s, ACTF.Ln)
            nc.scalar.activation(ps, lse, ACTF.Identity, bias=wneg)
            nc.scalar.dma_start(out_d[:], ps)

    # Route Exp, Ln (and Identity etc.) to the one table set containing
    # them all (natural_log_exp_and_others): pass the act tables in
    # original order (ids must stay act_info.json indices), empty set 0
    # (a non-empty set 0 attracts a redundant initial load), and drop the
    # combined set's functions from all other sets so the combined set is
    # always the first match.
    def _patched_act_loads():
        tabs = get_activation_tables(nc.m.arch)
        combined = tabs["natural_log_exp_and_others"]
        items = []
        for name, funcs in tabs.items():
            if name == "exp_and_others":
                funcs = set()
            elif name != "natural_log_exp_and_others":
                funcs = funcs - combined
            items.append((name, funcs))
        _bass_rust.insert_act_table_loads(nc, items)

    nc.insert_act_table_loads = _patched_act_loads
    nc.compile()
    return nc


_NC_CACHE = []


def _get_nc():
    if not _NC_CACHE:
        _NC_CACHE.append(_build())
    return _NC_CACHE[0]


def _make_in_maps(inputs, targets, postive_list):
    x = np.ascontiguousarray(np.asarray(inputs, dtype=np.float32))
    t = np.asarray(targets).astype(np.int64)
    p = np.asarray(postive_list).astype(np.int64)

    x8 = x.astype(ml_dtypes.float8_e4m3)

    # window slab + fused -1/(p+1) mask, from the full-precision input
    cols = np.arange(WIN, dtype=np.int64)
    idx = t[:, None] + cols[None, :]                    # [B, WIN]
    win = np.take_along_axis(x, idx, axis=1)            # [B, WIN] f32
    negmask = np.where(cols[None, :] <= p[:, None],
                       -1.0 / (p[:, None] + 1.0), 0.0).astype(np.float32)
    aux = np.concatenate([win, negmask], axis=1)        # [B, 2*WIN]

    in_maps = []
    for i in range(NCORES):
        sl = slice(i * BL, (i + 1) * BL)
        shard = x8[sl]
        parts, off = [], 0
        for e, w in SCHED:
            blk = shard[:, off:off + w]
            if e == "D":
                # [128 rows, w] -> [128 cols, w/128 tiles, 128 rows]
                blk = np.transpose(
                    blk.reshape(BL, w // BL, BL), (2, 1, 0))
            parts.append(np.ascontiguousarray(blk).reshape(-1))
            off += w
        in_maps.append({
            "x8": np.concatenate(parts),
            "aux": np.ascontiguousarray(aux[sl]),
        })
    return in_maps


def _run(inputs, targets, postive_list, trace=False, **kwargs):
    nc = _get_nc()
    in_maps = _make_in_maps(inputs, targets, postive_list)
    res = run_bass_kernel_spmd(nc, in_maps, core_ids=list(range(NCORES)),
                               trace=trace, **kwargs)
    total = np.float64(0.0)
    for i in range(NCORES):
        total += np.asarray(res.results[i]["ps"], dtype=np.float64).sum()
    value = np.float32(total / B)
    return value, res


def kernel(inputs, targets, postive_list):
    value, _ = _run(inputs, targets, postive_list, trace=False)
    return np.array(value, dtype=np.float32)


# revision 17
# speedup vs baseline: 1.1329x; 1.1327x over previous
"""Trainium2 Bass kernel for AudioToTextCrossEntropyLoss.

Math: loss = mean_b [ ln(sum_j exp(x_bj)) - (sum_{j=t_b}^{t_b+p_b} x_bj)/(p_b+1) ]

Sharding: data-parallel over batch — 1024 rows as 128 rows on each of 8
NeuronCores (rows on partitions). The kernel returns the 128 per-sample
losses per core; the host sums 1024 scalars and divides by 1024.

Per-core algorithm, designed around the engine rooflines:
  - x is sent as fp8_e4m3 (host cast): 4 MiB/core, ~13 us of DMA at the
    ~336 GB/s HBM cap instead of 47 us for f32. fp8 quantization of x
    perturbs ln-sum-exp by ~1e-4 rel.
  - The exp+row-sum work (the only O(B*N) compute) is split across THREE
    engines working on disjoint column ranges in parallel:
      * ScalarE (ACT): table exp with per-chunk accumulate (1 elem/cyc/lane)
        on row-major chunks.
      * VectorE (DVE): Schraudolph bit-trick exp — one 2-elem/cyc
        tensor_scalar computes i16(x*128/ln2 + bias), whose bit pattern
        read as bf16 is exp(x)*2^-16.
      * TensorE (PE): reduces the Schraudolph values — these chunks are
        sent TRANSPOSED (columns on partitions, rows on the free axis), so
        a ones-matmul contracts 128 columns at a time; 512-wide moving
        blocks accumulate into one PSUM bank across all chunks. Per-row
        sums land on the PSUM diagonal psum[r, g*128+r], extracted by one
        masked accumulate against an (r - p) iota.
    Chunks of the streams are interleaved in DMA order so all engines
    start ~1 us in and finish together.
  - The ragged window term needs only cols [t_b, t_b+p_b], p<=63: the host
    ships the 80-wide f32 slab at each row's t_b plus a fused f32 mask
    (-1/(p+1) inside the window, 0 outside); one DVE multiply-accumulate
    yields -window_mean. (Replaces 2 full masked passes over 16k cols.)
  - Final: s = sA + K*(sD_act + sD_pe) with K folding the 2^16 scale and
    the Schraudolph calibration; lse = Ln(s) and ps = Identity(lse + wneg)
    on ACT — the act-table pass is overridden so Exp, Ln and Identity all
    come from the natural_log_exp_and_others set: one table load, in the
    prologue. ps is DMA'd out per sample.
"""

import numpy as np
import ml_dtypes

import bass_rust as _bass_rust
import concourse.bacc as bacc
import concourse.mybir as mybir
import concourse.tile as tile
from concourse.bass_utils import run_bass_kernel_spmd
from concourse.hw_specs import get_activation_tables

F32 = mybir.dt.float32
BF16 = mybir.dt.bfloat16
FP8 = mybir.dt.float8e4
I16 = mybir.dt.int16
ALU = mybir.AluOpType
ACTF = mybir.ActivationFunctionType

B, N = 1024, 32768
NCORES = 8
BL = B // NCORES          # 128 rows per core
WIN = 80                  # window slab width (>= max p+1 = 64)
MMN = 512                 # moving free dim per matmul (PE max, 1 PSUM bank)

# Interleaved chunk schedule: (engine, width). "A" = ScalarE table exp on
# row-major chunks, "D" = DVE Schraudolph convert on transposed chunks
# reduced by PE. Small leading chunks start the engines early; widths are
# sized so all engines finish at ~the same time. D widths must be
# multiples of 512.
SCHED = [
    ("A", 1024), ("D", 2048), ("A", 4096), ("D", 4096),
    ("A", 4096), ("D", 4096), ("A", 4096), ("D", 4096),
    ("D", 4096), ("D", 1024),
]
assert sum(w for _, w in SCHED) == N
A_CHUNKS = [(i, w) for i, (e, w) in enumerate(SCHED) if e == "A"]
D_CHUNKS = [(i, w) for i, (e, w) in enumerate(SCHED) if e == "D"]
AMAX = max(w for _, w in A_CHUNKS)
DMAX = max(w for _, w in D_CHUNKS)
assert all(w % MMN == 0 for _, w in D_CHUNKS)

# Schraudolph constants: i16 value e*128+m bitcast as bf16 reads as
# (1+m/128)*2^(e-127) ~= exp(x)*2^-S when t = x*128/ln2 + (127-S)*128 - C.
# C zeroes the mean of the (1+f)*2^-f - 1 mantissa error; +0.5 centers the
# float->int conversion. CALIB (measured against the HW conversion/rounding
# behavior on fp8-quantized N(0,1)) removes the residual bias; K folds it
# with the 2^S scale.
SCALE_S = 16
A_CONST = 128.0 / float(np.log(2.0))
B_CONST = (127 - SCALE_S) * 128.0 - 7.21927 + 0.5
K_COMB = float(2.0 ** SCALE_S) * 0.99601


def _build():
    nc = bacc.Bacc("TRN2", target_bir_lowering=False, debug=False,
                   num_devices=NCORES)
    # x8 is chunk-major in SCHED order: "A" chunks are row-major [128, w]
    # blocks, "D" chunks are transposed [128 cols, w/128 tiles, 128 rows]
    # blocks — either way contiguous, so the shard streams from DRAM in
    # sequential address order.
    x_d = nc.dram_tensor("x8", [BL * N], FP8, kind="ExternalInput").ap()
    # cols 0..WIN-1: window values (f32); cols WIN..2*WIN-1: -1/(p+1) mask
    aux_d = nc.dram_tensor("aux", [BL, 2 * WIN], F32,
                           kind="ExternalInput").ap()
    out_d = nc.dram_tensor("ps", [BL, 1], F32, kind="ExternalOutput").ap()

    a_tot = sum(w for _, w in A_CHUNKS)
    d_tot = sum(w for _, w in D_CHUNKS)
    n_mm = d_tot // MMN

    with tile.TileContext(nc) as tc:
        with (
            tc.tile_pool(name="xp", bufs=1) as xpool,
            tc.tile_pool(name="dumps", bufs=1) as dumps,
            tc.tile_pool(name="small", bufs=1) as small,
            tc.tile_pool(name="ps", bufs=1, space="PSUM") as pspool,
        ):
            xa = xpool.tile([BL, a_tot], FP8, tag="xa")
            xd = xpool.tile([BL, d_tot], FP8, tag="xd")
            aux = small.tile([BL, 2 * WIN], F32, tag="aux")
            ones_t = small.tile([BL, BL], BF16, tag="ones")
            iota_t = small.tile([BL, MMN], F32, tag="iota")
            pA = small.tile([BL, len(A_CHUNKS)], F32, tag="pA")
            fin = small.tile([BL, 8], F32, tag="fin")
            ti0 = dumps.tile([BL, DMAX], I16, tag="ti0")
            ti1 = dumps.tile([BL, DMAX], I16, tag="ti1")
            ti2 = dumps.tile([BL, DMAX], I16, tag="ti2")
            expd = dumps.tile([BL, AMAX], FP8, tag="expd")
            wdump = dumps.tile([BL, WIN], F32, tag="wdump")
            ddump = dumps.tile([BL, MMN], F32, tag="ddump")
            psum = pspool.tile([BL, MMN], F32, tag="psum")

            wneg = fin[:, 0:1]    # -window_sum/(p+1)
            rA = fin[:, 1:2]      # sum of ACT partials
            rD = fin[:, 2:3]      # sum of PE-reduced Schraudolph values
            s = fin[:, 3:4]       # total sum of exp
            lse = fin[:, 4:5]
            ps = fin[:, 5:6]      # per-sample loss

            # Prologue constants on otherwise-idle engines: the (r - p)
            # iota for the PSUM diagonal extraction, and the all-ones
            # matmul weights.
            nc.gpsimd.iota(iota_t[:], pattern=[[0, MMN // BL], [1, BL]],
                           base=0, channel_multiplier=-1,
                           allow_small_or_imprecise_dtypes=True)
            nc.gpsimd.memset(ones_t[:], 1.0)

            # Per-chunk offsets into the per-engine SBUF tiles
            a_off, d_off, offs = 0, 0, []
            for e, w in SCHED:
                if e == "A":
                    offs.append(a_off)
                    a_off += w
                else:
                    offs.append(d_off)
                    d_off += w

            # Prologue DMAs: first chunk + aux on the scalar ring (issues
            # ~2 us before the sync ring's first issue), rest on sync.
            flat = 0
            for c, (e, w) in enumerate(SCHED):
                src = x_d[flat:flat + w * BL].rearrange("(p w) -> p w", p=BL)
                flat += w * BL
                dst = xa if e == "A" else xd
                eng = nc.scalar if c == 0 else nc.sync
                eng.dma_start(dst[:, offs[c]:offs[c] + w], src)
                if c == 0:
                    nc.sync.dma_start(aux[:], aux_d[:])

            # Window term (DVE, runs as soon as aux lands):
            # wneg = sum(win * (-1/(p+1) masked))
            nc.vector.scalar_tensor_tensor(
                wdump[:], aux[:, 0:WIN], 1.0, aux[:, WIN:2 * WIN],
                op0=ALU.mult, op1=ALU.mult, accum_out=wneg)

            # ScalarE stream: table exp + accumulate per chunk
            for i, (c, w) in enumerate(A_CHUNKS):
                o = offs[c]
                nc.scalar.activation(expd[:, :w], xa[:, o:o + w], ACTF.Exp,
                                     accum_out=pA[:, i:i + 1])

            # DVE + PE streams: Schraudolph convert (2 elem/cyc), then the
            # PE contracts each 128-column tile with ones weights, 512-wide
            # moving blocks, accumulating every block into the same PSUM
            # bank. ti ping-pongs so chunk c+1's convert overlaps chunk c's
            # matmuls.
            mm = 0
            for j, (c, w) in enumerate(D_CHUNKS):
                o = offs[c]
                ti = (ti0, ti1, ti2)[j % 3]
                nc.vector.tensor_scalar(ti[:, :w], xd[:, o:o + w],
                                        A_CONST, B_CONST,
                                        op0=ALU.mult, op1=ALU.add)
                for g in range(w // MMN):
                    nc.tensor.matmul(
                        psum[:], ones_t[:],
                        ti[:, g * MMN:(g + 1) * MMN].bitcast(BF16),
                        start=(mm == 0), stop=(mm == n_mm - 1))
                    mm += 1

            # psum[p, g*128+r] holds sum-of-group-g-tiles for row r; the
            # row-r total needs the diagonal r == p of each group:
            # rD = sum_f psum[p, f] * (iota[p, f] == 0)
            nc.vector.scalar_tensor_tensor(
                ddump[:], iota_t[:], 0.0, psum[:],
                op0=ALU.is_equal, op1=ALU.mult, accum_out=rD)

            # Combine: s = sum(pA) + K * rD; lse = Ln(s) and the final add
            # on ACT (same act table set as Exp — no mid-kernel load)
            nc.vector.tensor_reduce(rA, pA[:], axis=mybir.AxisListType.X,
                                    op=ALU.add)
            nc.vector.scalar_tensor_tensor(s, rD, K_COMB, rA,
                                           op0=ALU.mult, op1=ALU.add)
            nc.scalar.activation(lse, s, ACTF.Ln)
            nc.scalar.activation(ps, lse, ACTF.Identity, bias=wneg)
            nc.scalar.dma_start(out_d[:], ps)

    # Route Exp, Ln (and Identity etc.) to the one table set containing
    # them all (natural_log_exp_and_others): pass the act tables in
    # original order (ids must stay act_info.json indices), empty set 0
    # (a non-empty set 0 attracts a redundant initial load), and drop the
    # combined set's functions from all other sets so the combined set is
    # always the first match.
    def _patched_act_loads():
        tabs = get_activation_tables(nc.m.arch)
        combined = tabs["natural_log_exp_and_others"]
        items = []
        for name, funcs in tabs.items():
            if name == "exp_and_others":
                funcs = set()
            elif name != "natural_log_exp_and_others":
                funcs = funcs - combined
            items.append((name, funcs))
        _bass_rust.insert_act_table_loads(nc, items)

    nc.insert_act_table_loads = _patched_act_loads
    nc.compile()
    return nc


_NC_CACHE = []


def _get_nc():
    if not _NC_CACHE:
        _NC_CACHE.append(_build())
    return _NC_CACHE[0]


def _make_in_maps(inputs, targets, postive_list):
    x = np.ascontiguousarray(np.asarray(inputs, dtype=np.float32))
    t = np.asarray(targets).astype(np.int64)
    p = np.asarray(postive_list).astype(np.int64)

    x8 = x.astype(ml_dtypes.float8_e4m3)

    # window slab + fused -1/(p+1) mask, from the full-precision input
    cols = np.arange(WIN, dtype=np.int64)
    idx = t[:, None] + cols[None, :]                    # [B, WIN]
    win = np.take_along_axis(x, idx, axis=1)            # [B, WIN] f32
    negmask = np.where(cols[None, :] <= p[:, None],
                       -1.0 / (p[:, None] + 1.0), 0.0).astype(np.float32)
    aux = np.concatenate([win, negmask], axis=1)        # [B, 2*WIN]

    in_maps = []
    for i in range(NCORES):
        sl = slice(i * BL, (i + 1) * BL)
        shard = x8[sl]
        parts, off = [], 0
        for e, w in SCHED:
            blk = shard[:, off:off + w]
            if e == "D":
                # [128 rows, w] -> [128 cols, w/128 tiles, 128 rows]
                blk = np.transpose(
                    blk.reshape(BL, w // BL, BL), (2, 1, 0))
            parts.append(np.ascontiguousarray(blk).reshape(-1))
            off += w
        in_maps.append({
            "x8": np.concatenate(parts),
            "aux": np.ascontiguousarray(aux[sl]),
        })
    return in_maps


def _run(inputs, targets, postive_list, trace=False, **kwargs):
    nc = _get_nc()
    in_maps = _make_in_maps(inputs, targets, postive_list)
    res = run_bass_kernel_spmd(nc, in_maps, core_ids=list(range(NCORES)),
                               trace=trace, **kwargs)
    total = np.float64(0.0)
    for i in range(NCORES):
        total += np.asarray(res.results[i]["ps"], dtype=np.float64).sum()
    value = np.float32(total / B)
    return value, res


def kernel(inputs, targets, postive_list):
    value, _ = _run(inputs, targets, postive_list, trace=False)
    return np.array(value, dtype=np.float32)


# revision 20
# speedup vs baseline: 1.1334x; 1.0004x over previous
"""Trainium2 Bass kernel for AudioToTextCrossEntropyLoss.

Math: loss = mean_b [ ln(sum_j exp(x_bj)) - (sum_{j=t_b}^{t_b+p_b} x_bj)/(p_b+1) ]

Sharding: data-parallel over batch — 1024 rows as 128 rows on each of 8
NeuronCores (rows on partitions). The kernel returns the 128 per-sample
losses per core; the host sums 1024 scalars and divides by 1024.

Per-core algorithm, designed around the engine rooflines:
  - x is sent as fp8_e4m3 (host cast): 4 MiB/core, ~13 us of DMA at the
    ~336 GB/s HBM cap instead of 47 us for f32. fp8 quantization of x
    perturbs ln-sum-exp by ~1e-4 rel.
  - The exp+row-sum work (the only O(B*N) compute) is split across THREE
    engines working on disjoint column ranges in parallel:
      * ScalarE (ACT): table exp with per-chunk accumulate (1 elem/cyc/lane)
        on row-major chunks.
      * VectorE (DVE): Schraudolph bit-trick exp — one 2-elem/cyc
        tensor_scalar computes i16(x*128/ln2 + bias), whose bit pattern
        read as bf16 is exp(x)*2^-16.
      * TensorE (PE): reduces the Schraudolph values — these chunks are
        sent TRANSPOSED (columns on partitions, rows on the free axis), so
        a ones-matmul contracts 128 columns at a time; 512-wide moving
        blocks accumulate into one PSUM bank across all chunks. Per-row
        sums land on the PSUM diagonal psum[r, g*128+r], extracted by one
        masked accumulate against an (r - p) iota.
    Chunks of the streams are interleaved in DMA order so all engines
    start ~1 us in and finish together.
  - The ragged window term needs only cols [t_b, t_b+p_b], p<=63: the host
    ships the 80-wide f32 slab at each row's t_b plus a fused f32 mask
    (-1/(p+1) inside the window, 0 outside); one DVE multiply-accumulate
    yields -window_mean. (Replaces 2 full masked passes over 16k cols.)
  - Final: s = sA + K*(sD_act + sD_pe) with K folding the 2^16 scale and
    the Schraudolph calibration; lse = Ln(s) and ps = Identity(lse + wneg)
    on ACT — the act-table pass is overridden so Exp, Ln and Identity all
    come from the natural_log_exp_and_others set: one table load, in the
    prologue. ps is DMA'd out per sample.
"""

import numpy as np
import ml_dtypes

import bass_rust as _bass_rust
import concourse.bacc as bacc
import concourse.mybir as mybir
import concourse.tile as tile
from concourse.bass_utils import run_bass_kernel_spmd
from concourse.hw_specs import get_activation_tables

F32 = mybir.dt.float32
BF16 = mybir.dt.bfloat16
FP8 = mybir.dt.float8e4
I16 = mybir.dt.int16
ALU = mybir.AluOpType
ACTF = mybir.ActivationFunctionType

B, N = 1024, 32768
NCORES = 8
BL = B // NCORES          # 128 rows per core
WIN = 80                  # window slab width (>= max p+1 = 64)
MMN = 512                 # moving free dim per matmul (PE max, 1 PSUM bank)

# Interleaved chunk schedule: (engine, width). "A" = ScalarE table exp on
# row-major chunks, "D" = DVE Schraudolph convert on transposed chunks
# reduced by PE. Small leading chunks start the engines early; widths are
# sized so all engines finish at ~the same time. D widths must be
# multiples of 512.
SCHED = [
    ("A", 1024), ("D", 2048), ("A", 4096), ("D", 4096),
    ("A", 4096), ("D", 4096), ("A", 4096), ("D", 4096),
    ("D", 4096), ("D", 1024),
]
assert sum(w for _, w in SCHED) == N
A_CHUNKS = [(i, w) for i, (e, w) in enumerate(SCHED) if e == "A"]
D_CHUNKS = [(i, w) for i, (e, w) in enumerate(SCHED) if e == "D"]
AMAX = max(w for _, w in A_CHUNKS)
DMAX = max(w for _, w in D_CHUNKS)
assert all(w % MMN == 0 for _, w in D_CHUNKS)

# Schraudolph constants: i16 value e*128+m bitcast as bf16 reads as
# (1+m/128)*2^(e-127) ~= exp(x)*2^-S when t = x*128/ln2 + (127-S)*128 - C.
# C zeroes the mean of the (1+f)*2^-f - 1 mantissa error; +0.5 centers the
# float->int conversion. CALIB (measured against the HW conversion/rounding
# behavior on fp8-quantized N(0,1)) removes the residual bias; K folds it
# with the 2^S scale.
SCALE_S = 16
A_CONST = 128.0 / float(np.log(2.0))
B_CONST = (127 - SCALE_S) * 128.0 - 7.21927 + 0.5
K_COMB = float(2.0 ** SCALE_S) * 0.99601


def _build():
    nc = bacc.Bacc("TRN2", target_bir_lowering=False, debug=False,
                   num_devices=NCORES)
    # x8 is chunk-major in SCHED order: "A" chunks are row-major [128, w]
    # blocks, "D" chunks are transposed [128 cols, w/128 tiles, 128 rows]
    # blocks — either way contiguous, so the shard streams from DRAM in
    # sequential address order.
    x_d = nc.dram_tensor("x8", [BL * N], FP8, kind="ExternalInput").ap()
    # cols 0..WIN-1: window values (f32); cols WIN..2*WIN-1: -1/(p+1) mask
    aux_d = nc.dram_tensor("aux", [BL, 2 * WIN], F32,
                           kind="ExternalInput").ap()
    out_d = nc.dram_tensor("ps", [BL, 1], F32, kind="ExternalOutput").ap()

    a_tot = sum(w for _, w in A_CHUNKS)
    d_tot = sum(w for _, w in D_CHUNKS)
    n_mm = d_tot // MMN

    with tile.TileContext(nc) as tc:
        with (
            tc.tile_pool(name="xp", bufs=1) as xpool,
            tc.tile_pool(name="dumps", bufs=1) as dumps,
            tc.tile_pool(name="small", bufs=1) as small,
            tc.tile_pool(name="ps", bufs=1, space="PSUM") as pspool,
        ):
            xa = xpool.tile([BL, a_tot], FP8, tag="xa")
            xd = xpool.tile([BL, d_tot], FP8, tag="xd")
            aux = small.tile([BL, 2 * WIN], F32, tag="aux")
            ones_t = small.tile([BL, BL], BF16, tag="ones")
            iota_t = small.tile([BL, MMN], F32, tag="iota")
            pA = small.tile([BL, len(A_CHUNKS)], F32, tag="pA")
            fin = small.tile([BL, 8], F32, tag="fin")
            ti0 = dumps.tile([BL, DMAX], I16, tag="ti0")
            ti1 = dumps.tile([BL, DMAX], I16, tag="ti1")
            ti2 = dumps.tile([BL, DMAX], I16, tag="ti2")
            expd = dumps.tile([BL, AMAX], FP8, tag="expd")
            wdump = dumps.tile([BL, WIN], F32, tag="wdump")
            ddump = dumps.tile([BL, MMN], F32, tag="ddump")
            ddump2 = dumps.tile([BL, MMN], F32, tag="ddump2")
            psumA = pspool.tile([BL, MMN], F32, tag="psumA")
            psumB = pspool.tile([BL, MMN], F32, tag="psumB")

            wneg = fin[:, 0:1]    # -window_sum/(p+1)
            rA = fin[:, 1:2]      # sum of ACT partials
            rDa = fin[:, 2:3]     # PE-reduced Schraudolph values, bank A
            rDb = fin[:, 3:4]     # ... bank B
            rD = fin[:, 4:5]
            s = fin[:, 5:6]       # total sum of exp
            lse = fin[:, 6:7]
            ps = fin[:, 7:8]      # per-sample loss

            # Prologue constants on otherwise-idle engines: the (r - p)
            # iota for the PSUM diagonal extraction, and the all-ones
            # matmul weights.
            nc.gpsimd.iota(iota_t[:], pattern=[[0, MMN // BL], [1, BL]],
                           base=0, channel_multiplier=-1,
                           allow_small_or_imprecise_dtypes=True)
            nc.gpsimd.memset(ones_t[:], 1.0)

            # Per-chunk offsets into the per-engine SBUF tiles
            a_off, d_off, offs = 0, 0, []
            for e, w in SCHED:
                if e == "A":
                    offs.append(a_off)
                    a_off += w
                else:
                    offs.append(d_off)
                    d_off += w

            # Prologue DMAs: first chunk + aux on the scalar ring (issues
            # ~2 us before the sync ring's first issue), rest on sync.
            flat = 0
            for c, (e, w) in enumerate(SCHED):
                src = x_d[flat:flat + w * BL].rearrange("(p w) -> p w", p=BL)
                flat += w * BL
                dst = xa if e == "A" else xd
                eng = nc.scalar if c == 0 else nc.sync
                eng.dma_start(dst[:, offs[c]:offs[c] + w], src)
                if c == 0:
                    nc.sync.dma_start(aux[:], aux_d[:])

            # Window term (DVE, runs as soon as aux lands):
            # wneg = sum(win * (-1/(p+1) masked))
            nc.vector.scalar_tensor_tensor(
                wdump[:], aux[:, 0:WIN], 1.0, aux[:, WIN:2 * WIN],
                op0=ALU.mult, op1=ALU.mult, accum_out=wneg)

            # ScalarE stream: table exp + accumulate per chunk
            for i, (c, w) in enumerate(A_CHUNKS):
                o = offs[c]
                nc.scalar.activation(expd[:, :w], xa[:, o:o + w], ACTF.Exp,
                                     accum_out=pA[:, i:i + 1])

            # DVE + PE streams: Schraudolph convert (2 elem/cyc), then the
            # PE contracts each 128-column tile with ones weights, 512-wide
            # moving blocks, accumulating every block into the same PSUM
            # bank. ti ping-pongs so chunk c+1's convert overlaps chunk c's
            # matmuls.
            # Matmuls alternate between two PSUM banks so consecutive
            # accumulates don't read-modify-write the same bank.
            mm = 0
            for j, (c, w) in enumerate(D_CHUNKS):
                o = offs[c]
                ti = (ti0, ti1, ti2)[j % 3]
                nc.vector.tensor_scalar(ti[:, :w], xd[:, o:o + w],
                                        A_CONST, B_CONST,
                                        op0=ALU.mult, op1=ALU.add)
                for g in range(w // MMN):
                    psum = (psumA, psumB)[mm % 2]
                    nc.tensor.matmul(
                        psum[:], ones_t[:],
                        ti[:, g * MMN:(g + 1) * MMN].bitcast(BF16),
                        start=(mm < 2), stop=(mm >= n_mm - 2))
                    mm += 1

            # psum[p, g*128+r] holds sum-of-group-g-tiles for row r; the
            # row-r total needs the diagonal r == p of each group:
            # rD = sum_f psum[p, f] * (iota[p, f] == 0), over both banks
            nc.vector.scalar_tensor_tensor(
                ddump[:], iota_t[:], 0.0, psumA[:],
                op0=ALU.is_equal, op1=ALU.mult, accum_out=rDa)
            nc.vector.scalar_tensor_tensor(
                ddump2[:], iota_t[:], 0.0, psumB[:],
                op0=ALU.is_equal, op1=ALU.mult, accum_out=rDb)

            # Combine: s = sum(pA) + K * rD; lse = Ln(s) and the final add
            # on ACT (same act table set as Exp — no mid-kernel load)
            nc.vector.tensor_reduce(rA, pA[:], axis=mybir.AxisListType.X,
                                    op=ALU.add)
            nc.vector.tensor_tensor(rD, rDa, rDb, op=ALU.add)
            nc.vector.scalar_tensor_tensor(s, rD, K_COMB, rA,
                                           op0=ALU.mult, op1=ALU.add)
            nc.scalar.activation(lse, s, ACTF.Ln)
            nc.scalar.activation(ps, lse, ACTF.Identity, bias=wneg)
            nc.scalar.dma_start(out_d[:], ps)

    # Route Exp, Ln (and Identity etc.) to the one table set containing
    # them all (natural_log_exp_and_others): pass the act tables in
    # original order (ids must stay act_info.json indices), empty set 0
    # (a non-empty set 0 attracts a redundant initial load), and drop the
    # combined set's functions from all other sets so the combined set is
    # always the first match.
    def _patched_act_loads():
        tabs = get_activation_tables(nc.m.arch)
        combined = tabs["natural_log_exp_and_others"]
        items = []
        for name, funcs in tabs.items():
            if name == "exp_and_others":
                funcs = set()
            elif name != "natural_log_exp_and_others":
                funcs = funcs - combined
            items.append((name, funcs))
        _bass_rust.insert_act_table_loads(nc, items)

    nc.insert_act_table_loads = _patched_act_loads
    nc.compile()
    return nc


_NC_CACHE = []


def _get_nc():
    if not _NC_CACHE:
        _NC_CACHE.append(_build())
    return _NC_CACHE[0]


def _make_in_maps(inputs, targets, postive_list):
    x = np.ascontiguousarray(np.asarray(inputs, dtype=np.float32))
    t = np.asarray(targets).astype(np.int64)
    p = np.asarray(postive_list).astype(np.int64)

    x8 = x.astype(ml_dtypes.float8_e4m3)

    # window slab + fused -1/(p+1) mask, from the full-precision input
    cols = np.arange(WIN, dtype=np.int64)
    idx = t[:, None] + cols[None, :]                    # [B, WIN]
    win = np.take_along_axis(x, idx, axis=1)            # [B, WIN] f32
    negmask = np.where(cols[None, :] <= p[:, None],
                       -1.0 / (p[:, None] + 1.0), 0.0).astype(np.float32)
    aux = np.concatenate([win, negmask], axis=1)        # [B, 2*WIN]

    in_maps = []
    for i in range(NCORES):
        sl = slice(i * BL, (i + 1) * BL)
        shard = x8[sl]
        parts, off = [], 0
        for e, w in SCHED:
            blk = shard[:, off:off + w]
            if e == "D":
                # [128 rows, w] -> [128 cols, w/128 tiles, 128 rows]
                blk = np.transpose(
                    blk.reshape(BL, w // BL, BL), (2, 1, 0))
            parts.append(np.ascontiguousarray(blk).reshape(-1))
            off += w
        in_maps.append({
            "x8": np.concatenate(parts),
            "aux": np.ascontiguousarray(aux[sl]),
        })
    return in_maps


def _run(inputs, targets, postive_list, trace=False, **kwargs):
    nc = _get_nc()
    in_maps = _make_in_maps(inputs, targets, postive_list)
    res = run_bass_kernel_spmd(nc, in_maps, core_ids=list(range(NCORES)),
                               trace=trace, **kwargs)
    total = np.float64(0.0)
    for i in range(NCORES):
        total += np.asarray(res.results[i]["ps"], dtype=np.float64).sum()
    value = np.float32(total / B)
    return value, res


def kernel(inputs, targets, postive_list):
    value, _ = _run(inputs, targets, postive_list, trace=False)
    return np.array(value, dtype=np.float32)
